# revision 1
# baseline (speedup 1.0000x reference)
"""Trainium2 Bass kernel for differentiable KDE (Gaussian kernel density estimate).

Math (h = 1):
    sq[i,j]    = ||x_i||^2 + ||d_j||^2 - 2 x_i.d_j
    density[i] = mean_j exp(-C * sq[i,j]),   C = 0.5 / sqrt(2*pi)
               = exp(-C||x_i||^2 - ln M) * sum_j exp(2C x_i.d_j - C||d_j||^2)

Sharding: data-parallel over x rows (1024 per core), data replicated.

Per-core pipeline (j = data row as PSUM partition, i = x row as free dim):
    - DMA data in 8 row-interleaved chunks; per 128-row tile: DVE
      square+reduce -> ||d_j||^2 bias column, PE transpose -> dataT in SBUF
      as float32r (tf32-grade matmul dtype, full-rate weight streaming).
    - Main loop over 64 j-tiles: PE matmul psum[j=128, i=1024] =
      dataT_jt.T @ xT (float32r), ACT exp with per-partition bias
      -C||d_j||^2 and scale 2C -> E (float32r), PE matvec with all-ones
      stationary accumulates sum_j E over all 64 j-tiles into two
      persistent PSUM banks [1, 512].
    - Epilogue: density = acc * exp(-C||x_i||^2 - ln M) (norms via squared
      transposed x + ones-matvec so the factor lands in [1, 1024] layout).
"""
import math
from contextlib import ExitStack

import numpy as np

from concourse import bacc, mybir, tile
from concourse.bass_utils import run_bass_kernel_spmd
from concourse import masks

N, M, D = 8192, 8192, 128
NCORES = 8
NS = N // NCORES            # 1024 x-rows per core
P = 128                     # partitions
NT_X = NS // P              # 8 x tiles
NT_D = M // P               # 64 data tiles
NCHUNK = 8                  # data DMA chunks
TPC = NT_D // NCHUNK        # 8 tiles per chunk

C = 0.5 / math.sqrt(2.0 * math.pi)          # 0.19947114020071635
TWO_C = 2.0 * C                             # 0.3989422804014327
LNM = math.log(float(M))                    # ln 8192

F32 = mybir.dt.float32
F32R = mybir.dt.float32r

_CACHED_NC = None


def _build():
    nc = bacc.Bacc("TRN2", target_bir_lowering=False, debug=False)
    x_d = nc.dram_tensor("x", [NS, D], F32, kind="ExternalInput")
    d_d = nc.dram_tensor("data", [M, D], F32, kind="ExternalInput")
    o_d = nc.dram_tensor("out", [1, NS], F32, kind="ExternalOutput")

    # x loads contiguously (one 4KB packet per partition; row p*8+r lands at
    # [p, r]) — the induced permutation of xT columns is undone by one
    # on-chip reorder copy of the [1, 1024] result at the end.
    x_re = x_d.ap().rearrange("(p r) d -> p r d", p=P)     # [128, 8, 128]
    d_re = d_d.ap().rearrange("(s p) d -> p s d", p=P)     # [128, 64, 128]

    with tile.TileContext(nc) as tc, ExitStack() as ctx:
        const_pool = ctx.enter_context(tc.tile_pool(name="const", bufs=1))
        dT_pool = ctx.enter_context(tc.tile_pool(name="dT", bufs=1))
        xbuf_pool = ctx.enter_context(tc.tile_pool(name="xbuf", bufs=1))
        drow_pool = ctx.enter_context(tc.tile_pool(name="drow", bufs=4))
        scr_pool = ctx.enter_context(tc.tile_pool(name="scr", bufs=2))
        e_pool = ctx.enter_context(tc.tile_pool(name="e", bufs=3))
        out_pool = ctx.enter_context(tc.tile_pool(name="outp", bufs=1))
        ps_main = ctx.enter_context(tc.tile_pool(name="psm", bufs=2, space="PSUM"))
        ps_acc = ctx.enter_context(tc.tile_pool(name="psa", bufs=1, space="PSUM"))
        ps_tr = ctx.enter_context(tc.tile_pool(name="pst", bufs=2, space="PSUM"))

        ident = const_pool.tile([P, P], F32, tag="ident")
        masks.make_identity(nc, ident[:])
        ones_f = const_pool.tile([P, 1], F32, tag="onesf")
        nc.gpsimd.memset(ones_f[:], 1.0)
        ones_r = const_pool.tile([P, 1], F32R, tag="ones")
        nc.vector.tensor_copy(ones_r[:], ones_f[:])
        nlm_bias = const_pool.tile([1, 1], F32, tag="nlm")
        nc.gpsimd.memset(nlm_bias[:], -LNM)

        dataT = dT_pool.tile([P, M], F32R, tag="dataT")          # 32KB/part
        xT = xbuf_pool.tile([P, NS], F32R, tag="xT")
        xsqT = xbuf_pool.tile([P, NS], F32R, tag="xsqT")
        xrow = xbuf_pool.tile([P, NT_X, P], F32, tag="xrow")
        dnsq = const_pool.tile([P, NT_D], F32, tag="dnsq")
        dbias = const_pool.tile([P, NT_D], F32, tag="dbias")
        exf = out_pool.tile([1, NS], F32, tag="exf")
        dens = out_pool.tile([1, NS], F32, tag="dens")

        # ---- x prologue: load, transpose, squared-norm factor in [1, NS] ----
        # contiguous x is tiny (128 packets) — put it FIRST on the sync queue
        nc.sync.dma_start(xrow[:], x_re)
        for t in range(NT_X):
            tr = ps_tr.tile([P, P], F32, tag="tr")
            nc.tensor.transpose(tr[:], xrow[:, t, :], ident[:])
            nc.vector.tensor_copy(xT[:, t * P:(t + 1) * P], tr[:])
        nc.vector.tensor_mul(xsqT[:], xT[:].bitcast(F32), xT[:].bitcast(F32))
        pmx = ps_main.tile([P, NS], F32, tag="pm")
        for c2 in range(2):
            sl = slice(c2 * 512, (c2 + 1) * 512)
            nc.tensor.matmul(pmx[0:1, sl], ones_r[:], xsqT[:, sl],
                             start=True, stop=True)
        nc.scalar.activation(exf[:], pmx[0:1, :],
                             mybir.ActivationFunctionType.Exp,
                             bias=nlm_bias[:], scale=-C)

        # ---- data prologue: stream chunks; norms + transposes per tile ----
        for ch in range(NCHUNK):
            drow = drow_pool.tile([P, TPC, P], F32, tag="drow")
            nc.sync.dma_start(drow[:], d_re[:, ch * TPC:(ch + 1) * TPC, :])
            for k in range(TPC):
                s = ch * TPC + k
                scr = scr_pool.tile([P, P], F32, tag="scr")
                nc.vector.tensor_mul(scr[:], drow[:, k, :], drow[:, k, :])
                nc.vector.tensor_reduce(
                    dnsq[:, s:s + 1], scr[:],
                    axis=mybir.AxisListType.X, op=mybir.AluOpType.add)
                tr = ps_tr.tile([P, P], F32, tag="tr")
                nc.tensor.transpose(tr[:], drow[:, k, :], ident[:])
                nc.vector.tensor_copy(dataT[:, s * P:(s + 1) * P], tr[:])
            csl = slice(ch * TPC, (ch + 1) * TPC)
            nc.vector.tensor_scalar_mul(dbias[:, csl], dnsq[:, csl], -C)

        # ---- main loop over data tiles ----
        acc0 = ps_acc.tile([1, 512], F32, tag="acc0")
        acc1 = ps_acc.tile([1, 512], F32, tag="acc1")
        for jt in range(NT_D):
            pm = ps_main.tile([P, NS], F32, tag="pm")
            dsl = dataT[:, jt * P:(jt + 1) * P]
            nc.tensor.matmul(pm[:, 0:512], dsl, xT[:, 0:512],
                             start=True, stop=True)
            nc.tensor.matmul(pm[:, 512:1024], dsl, xT[:, 512:1024],
                             start=True, stop=True)
            e = e_pool.tile([P, NS], F32R, tag="e")
            nc.scalar.activation(e[:], pm[:],
                                 mybir.ActivationFunctionType.Exp,
                                 bias=dbias[:, jt:jt + 1], scale=TWO_C)
            nc.tensor.matmul(acc0[:], ones_r[:], e[:, 0:512],
                             start=(jt == 0), stop=(jt == NT_D - 1),
                             skip_group_check=True)
            nc.tensor.matmul(acc1[:], ones_r[:], e[:, 512:1024],
                             start=(jt == 0), stop=(jt == NT_D - 1),
                             skip_group_check=True)

        # ---- epilogue ----
        nc.vector.tensor_mul(dens[:, 0:512], acc0[:], exf[:, 0:512])
        nc.vector.tensor_mul(dens[:, 512:1024], acc1[:], exf[:, 512:1024])
        # undo the x row permutation: dens index r*128+p -> row 8p+r
        dens_o = out_pool.tile([1, NS], F32, tag="dens_o")
        nc.vector.tensor_copy(
            dens_o[:], dens[:].rearrange("o (r p) -> o p r", p=P))
        nc.sync.dma_start(o_d.ap(), dens_o[:])

    nc.compile()
    return nc


def kernel(x, data):
    global _CACHED_NC
    x = np.ascontiguousarray(np.asarray(x, dtype=np.float32))
    data = np.ascontiguousarray(np.asarray(data, dtype=np.float32))
    assert x.shape == (N, D) and data.shape == (M, D)

    if _CACHED_NC is None:
        _CACHED_NC = _build()
    nc = _CACHED_NC

    in_maps = [
        {"x": x[c * NS:(c + 1) * NS], "data": data} for c in range(NCORES)
    ]
    res = run_bass_kernel_spmd(nc, in_maps, list(range(NCORES)))
    dens = np.concatenate(
        [np.asarray(res.results[c]["out"]).reshape(NS) for c in range(NCORES)]
    )
    return dens.reshape(N, 1).astype(np.float32)


if __name__ == "__main__":
    rng = np.random.default_rng(0)
    x = rng.standard_normal((N, D), dtype=np.float32)
    data = rng.standard_normal((M, D), dtype=np.float32)
    out = kernel(x, data)
    print("kernel out", out.shape, out[:4, 0])



# revision 7
# speedup vs baseline: 1.2249x; 1.2249x over previous
"""Trainium2 Bass kernel for differentiable KDE (Gaussian kernel density estimate).

Math (h = 1):
    density[i] = (1/M) * sum_j exp(-C * ||x_i - d_j||^2),  C = 0.5 / sqrt(2*pi)
               = exp(-C||x_i||^2)/M * sum_j exp(2C x_i.d_j - C||d_j||^2)

Sharding: 4 x-shards x 2 data-shards over 8 cores. Each core computes
    root[p, i] = sum_{t} exp(2C x_i . d_{j0+128t+p} - C||d_{j0+128t+p}||^2)
(the j-tile-partial sums, 128 partitions x 2048 x-columns, bf16) and the host
finishes: per-x-shard sum over the two j-shards' roots' partitions, times
exp(-C||x_i||^2)/M.

Per-core pipeline:
    - Host passes pre-transposed xT [128, 2048] f32 and dT [128, 4096] f32
      (contraction dim on partitions; no on-chip transposes) plus the
      per-j-tile bias table db [128, 32] = -C||d_j||^2.
    - 32 j-tiles: PE matmul psum[j=128, i=2048] = dT_tile.T @ xT (f32r,
      4x512-wide), ACT exp -> e bf16 [128, 2048] with per-partition bias,
      DVE pairwise-tree bf16 adds accumulate the 32 e-tiles into one root.
    - DMA root out; host reduces partitions and applies the x factor.
"""
import math

import numpy as np

from concourse import bacc, mybir, tile
from concourse.bass_utils import run_bass_kernel_spmd

N, M, D = 8192, 8192, 128
P = 128
GI, GJ = 4, 2               # core grid: 4 x-shards x 2 data-shards
NCORES = GI * GJ
NI = N // GI                # 2048 x-rows per core
MJ = M // GJ                # 4096 data rows per core
NTJ = MJ // P               # 32 j-tiles
NCHUNK = 8                  # dT DMA chunks
TPC = NTJ // NCHUNK         # 4 j-tiles per chunk
WC = NI // 512              # 512-wide matmul slices per psum tile

C = 0.5 / math.sqrt(2.0 * math.pi)
TWO_C = 2.0 * C

F32 = mybir.dt.float32
F32R = mybir.dt.float32r
BF16 = mybir.dt.bfloat16

_CACHED_NC = None


def _build():
    nc = bacc.Bacc("TRN2", target_bir_lowering=False, debug=False)
    x_d = nc.dram_tensor("xT", [P, NI], F32R, kind="ExternalInput")
    d_d = nc.dram_tensor("dT", [P, MJ], F32R, kind="ExternalInput")
    b_d = nc.dram_tensor("db", [P, NTJ], F32, kind="ExternalInput")
    o_d = nc.dram_tensor("root", [P, NI], BF16, kind="ExternalOutput")

    with tile.TileContext(nc) as tc:
        with tc.tile_pool(name="xbuf", bufs=1) as xbuf_pool, \
             tc.tile_pool(name="dbuf", bufs=NCHUNK) as dbuf_pool, \
             tc.tile_pool(name="bias", bufs=1) as bias_pool, \
             tc.tile_pool(name="e", bufs=7) as e_pool, \
             tc.tile_pool(name="lvl", bufs=1) as lvl_pool, \
             tc.tile_pool(name="psm", bufs=2, space="PSUM") as ps_main:

            xT = xbuf_pool.tile([P, NI], F32R, tag="xT")
            db = bias_pool.tile([P, NTJ], F32, tag="db")
            nc.sync.dma_start(xT[:], x_d.ap())
            nc.sync.dma_start(db[:], b_d.ap())

            chunks = []
            for ch in range(NCHUNK):
                dchunk = dbuf_pool.tile([P, TPC * P], F32R, tag="dch")
                nc.sync.dma_start(
                    dchunk[:], d_d.ap()[:, ch * TPC * P:(ch + 1) * TPC * P])
                chunks.append(dchunk)

            # 4 interleaved bf16 accumulators over the 32 e-tiles (keeps the
            # rounding chain short); merged at the end
            NACC = 4
            accs = []
            for k in range(NACC):
                acc_t = lvl_pool.tile([P, NI], BF16, tag=f"acc{k}")
                accs.append(acc_t)
            held = []  # first NACC leaves, pair-added when their partner lands

            for jt in range(NTJ):
                dsl = chunks[jt // TPC][:, (jt % TPC) * P:(jt % TPC + 1) * P]
                pm = ps_main.tile([P, NI], F32, tag="pm")
                for c in range(WC):
                    sl = slice(c * 512, (c + 1) * 512)
                    nc.tensor.matmul(pm[:, sl], dsl,
                                     xT[:, sl],
                                     start=True, stop=True)
                e = e_pool.tile([P, NI], BF16, tag="e")
                nc.scalar.activation(e[:], pm[:],
                                     mybir.ActivationFunctionType.Exp,
                                     bias=db[:, jt:jt + 1], scale=TWO_C)
                k = jt % NACC
                if jt < NACC:
                    held.append(e)
                elif jt < 2 * NACC:
                    nc.vector.tensor_add(accs[k][:], held[k][:], e[:])
                else:
                    nc.vector.tensor_add(accs[k][:], accs[k][:], e[:])

            # merge: acc0 += acc1, acc2 += acc3, acc0 += acc2
            nc.vector.tensor_add(accs[0][:], accs[0][:], accs[1][:])
            nc.vector.tensor_add(accs[2][:], accs[2][:], accs[3][:])
            nc.vector.tensor_add(accs[0][:], accs[0][:], accs[2][:])

            nc.sync.dma_start(o_d.ap(), accs[0][:])

    nc.compile()
    return nc


def make_in_maps(x, data):
    """Host prep: transpose/shard inputs. Returns (in_maps, xfac[N] f64)."""
    x = np.ascontiguousarray(np.asarray(x, dtype=np.float32))
    data = np.ascontiguousarray(np.asarray(data, dtype=np.float32))
    assert x.shape == (N, D) and data.shape == (M, D)

    xT = np.ascontiguousarray(x.T)                    # [128, N]
    dT = np.ascontiguousarray(data.T)                 # [128, M]
    dnsq = np.einsum("md,md->m", data.astype(np.float64),
                     data.astype(np.float64))         # [M]
    db_full = (-C * dnsq).reshape(M // P, P).T.astype(np.float32)  # [128, M/P]

    xnsq = np.einsum("nd,nd->n", x.astype(np.float64), x.astype(np.float64))
    xfac = np.exp(-C * xnsq) / float(M)               # [N] f64

    in_maps = []
    for c in range(NCORES):
        gi, gj = c // GJ, c % GJ
        in_maps.append({
            "xT": np.ascontiguousarray(xT[:, gi * NI:(gi + 1) * NI]),
            "dT": np.ascontiguousarray(dT[:, gj * MJ:(gj + 1) * MJ]),
            "db": np.ascontiguousarray(
                db_full[:, gj * NTJ:(gj + 1) * NTJ]),
        })
    return in_maps, xfac


def kernel(x, data):
    global _CACHED_NC
    if _CACHED_NC is None:
        _CACHED_NC = _build()
    nc = _CACHED_NC

    in_maps, xfac = make_in_maps(x, data)
    res = run_bass_kernel_spmd(nc, in_maps, list(range(NCORES)))

    dens = np.empty(N, dtype=np.float64)
    for gi in range(GI):
        s = np.zeros(NI, dtype=np.float64)
        for gj in range(GJ):
            root = np.asarray(res.results[gi * GJ + gj]["root"])
            s += root.astype(np.float64).sum(axis=0)
        sl = slice(gi * NI, (gi + 1) * NI)
        dens[sl] = s * xfac[sl]
    return dens.reshape(N, 1).astype(np.float32)


if __name__ == "__main__":
    rng = np.random.default_rng(0)
    x = rng.standard_normal((N, D), dtype=np.float32)
    data = rng.standard_normal((M, D), dtype=np.float32)
    out = kernel(x, data)
    print("kernel out", out.shape, out[:4, 0])
